# revision 7
# baseline (speedup 1.0000x reference)
"""PCEN kernel for Trainium2, sharded over the time axis across 8 NeuronCores.

Strategy:
  - data is [B=8, F=128, T=16384] fp32. Each core owns a T-slice of 2048 cols
    (all batches). Inputs are downcast to bf16 on the host and outputs are
    shipped back as bf16 (rel tolerance is 2e-2; bf16 rounding is ~2e-3 max),
    halving DMA traffic: ~8.5MB/core vs ~17MB -> DMA roofline ~22us.
  - The EMA smoother M uses only batch 0. Each core computes M for its own
    slice independently via a 32-col halo: contributions older than 32 steps
    are attenuated by (1-s)^32 ~ 2e-10, far below the 2e-2 tolerance. The
    scan itself is one native TensorTensorScan DVE instruction (fp32 state).
  - R = 1/M^alpha = exp(-alpha*ln(M+eps)) on the ACT engine (<=2 ULP tables;
    the Reciprocal activation is banned for accuracy).
  - Per batch b: E = x*R (DVE bf16 2x mode), U = sqrt(E + delta) (ACT fp32),
    out = U - delta^r (DVE tensor_scalar, fp32 in / bf16 out). The subtract
    must read fp32 U: near-zero outputs cancel (U ~ sqrt(delta)), so a bf16
    U would blow up the relative error. GPSIMD is excluded from the hot
    path: its tensor_scalar runs at ~9 G elem/s vs DVE's ~240 G elem/s.
  - All input DMAs are queued upfront on the sync (SP) HWDGE ring, x0-halo
    first then batches in compute order: the ring drains FIFO, so arrivals
    are staggered to match the compute pipeline instead of all completing
    together under round-robin.
"""

import sys

if "/opt/trn_rl_repo" not in sys.path:
    sys.path.insert(0, "/opt/trn_rl_repo")

from contextlib import ExitStack

import ml_dtypes
import numpy as np

import concourse.bass as bass
import concourse.mybir as mybir
import concourse.tile as tile
from concourse import bacc
from concourse.bass_utils import run_bass_kernel_spmd

B, F, T = 8, 128, 16384
NCORES = 8
TLOC = T // NCORES  # 2048 cols per core
HALO = 32  # scan warmup; (1-s)^32 ~ 2e-10 << 2e-2 tolerance
TH = TLOC + HALO
F32 = mybir.dt.float32
BF16 = mybir.dt.bfloat16
NPBF16 = ml_dtypes.bfloat16

_nc_cache: dict = {}


def build_nc(
    alpha: float,
    r: float,
    delta: float,
    s: float,
    eps: float,
    nbatch: int = B,
    tloc: int = TLOC,
    halo: int = HALO,
    reps: int = 1,
) -> bass.Bass:
    r_abs = abs(r)
    # constant subtracted at the end: delta ** |r|
    C = float(np.float32(delta) ** np.float32(r_abs))
    AF = mybir.ActivationFunctionType
    OP = mybir.AluOpType
    th = tloc + halo

    # Bacc (not raw Bass): its compile() lowers multi-sem waits into separate
    # sequencer instructions; the DMA/scan ISA structs hold only one wait.
    nc = bacc.Bacc("TRN2", target_bir_lowering=False, debug=False, num_devices=NCORES)

    xs = nc.dram_tensor("xs", [nbatch - 1, F, tloc], BF16, kind="ExternalInput").ap()
    x0h = nc.dram_tensor("x0h", [F, th], BF16, kind="ExternalInput").ap()
    out = nc.dram_tensor("out", [nbatch, F, tloc], BF16, kind="ExternalOutput").ap()
    # scratch target for the x0h-priority gate DMA (see _phase_ab)
    gate = nc.dram_tensor("gate", [F, 1], BF16, kind="ExternalOutput").ap()

    with ExitStack() as ctx:
        tc = ctx.enter_context(tile.TileContext(nc))
        scanp = ctx.enter_context(tc.tile_pool(name="scan", bufs=1))
        xpool = ctx.enter_context(tc.tile_pool(name="x", bufs=nbatch - 1))
        epool = ctx.enter_context(tc.tile_pool(name="e", bufs=3))
        upool = ctx.enter_context(tc.tile_pool(name="u", bufs=3))
        # one buf per batch: the out-DMA then has no slot-reuse (WAR) wait,
        # keeping it within the DMA struct's single sync-wait slot
        opool = ctx.enter_context(tc.tile_pool(name="o", bufs=nbatch))

        # reps>1 repeats the whole computation back-to-back (benchmarking
        # only: isolates device time from host/axon dispatch overhead)
        for _rep in range(reps):
            _phase_ab(nc, tc, scanp, xpool, epool, upool, opool,
                      xs, x0h, out, gate, nbatch, tloc, halo, th,
                      s, eps, alpha, delta, r_abs, C, AF, OP)
    nc.compile()
    return nc


def _phase_ab(nc, tc, scanp, xpool, epool, upool, opool,
              xs, x0h, out, gate, nbatch, tloc, halo, th,
              s, eps, alpha, delta, r_abs, C, AF, OP):
        # ---- input DMAs, x0h prioritized ----
        x0t = scanp.tile([F, th], BF16, tag="x0t")
        nc.sync.dma_start(x0t[:], x0h[:])
        # gate: a 1-col SBUF->DRAM copy of x0t makes the sync sequencer wait
        # for the x0h DMA to land before enqueueing the batch loads below, so
        # x0h gets the full SDMA bandwidth (the scan is the critical path)
        # instead of round-robining with 3.7MB of batch input.
        nc.sync.dma_start(gate[:], x0t[:, :1])
        xtiles = []
        for b in range(1, nbatch):
            xbt = xpool.tile([F, tloc], BF16, tag="xb")
            nc.sync.dma_start(xbt[:], xs[b - 1])
            xtiles.append(xbt)

        # per-kernel const-bias tiles (activation() requires non-Copy biases
        # as APs); tile-pool deps replace the global all_engine_barrier the
        # shared const-AP registry would need.
        eps_t = scanp.tile([F, 1], F32, tag="epsc")
        nc.vector.memset(eps_t[:], float(eps))
        delta_t = scanp.tile([F, 1], F32, tag="deltac")
        nc.vector.memset(delta_t[:], float(delta))

        # ---- Phase A: EMA scan on batch 0 slice (with halo) ----
        # single decay column broadcast along the free dim (step-0 AP):
        # replaces a [F, th] memset on the scan's critical path
        dcol = scanp.tile([F, 1], BF16, tag="dcol")
        nc.vector.memset(dcol[:], 1.0 - s)
        # 1-col probe copy on the vector engine carries the DMA-completion
        # wait, so the scan instruction itself needs no cross-engine sync
        # waits (the TensorTensorScan ISA struct has too few wait slots).
        probe = scanp.tile([F, 1], BF16, tag="probe")
        nc.vector.tensor_copy(probe[:], x0t[:, :1])
        m = scanp.tile([F, th], F32, tag="m")
        # scan on raw x: state = (1-s)*state + x  => true EMA m = s*state.
        # The s factor is folded into the Ln activation's input scale below.
        dbc, _ = bass.broadcast_tensor_aps(dcol[:], x0t[:])
        nc.vector.tensor_tensor_scan(m[:], dbc, x0t[:], 0.0, OP.mult, OP.add)
        # R = exp(-alpha * ln(s*m' + eps)) == (M+eps)^-alpha  (M+eps > 0 always)
        lnm = scanp.tile([F, tloc], F32, tag="lnm")
        nc.scalar.activation(lnm[:], m[:, halo:], AF.Ln, bias=eps_t[:], scale=float(s))
        rr = scanp.tile([F, tloc], BF16, tag="rr")
        nc.scalar.activation(rr[:], lnm[:], AF.Exp, scale=-float(alpha))

        # ---- Phase B: per-batch elementwise PCEN ----
        for b in range(nbatch):
            if b == 0:
                xb = x0t[:, halo:]  # batch 0 slice already on chip
            else:
                xb = xtiles[b - 1][:]
            e = epool.tile([F, tloc], BF16, tag="e")
            nc.vector.tensor_mul(e[:], xb, rr[:])  # E = x / M^alpha
            u = upool.tile([F, tloc], F32, tag="u")
            if r_abs == 0.5:
                nc.scalar.activation(u[:], e[:], AF.Sqrt, bias=delta_t[:])
            else:
                lne = upool.tile([F, tloc], F32, tag="lne")
                nc.scalar.activation(lne[:], e[:], AF.Ln, bias=delta_t[:])
                nc.scalar.activation(u[:], lne[:], AF.Exp, scale=float(r_abs))
            o = opool.tile([F, tloc], BF16, tag="o")
            if b == nbatch - 1:
                # last batch's subtract on ACT (Copy is table-free): balances
                # DVE (8 muls + 7 subs) vs ACT (8 sqrts + 1 copy) and avoids
                # a DVE->ACT->DVE ping-pong on the drain tail.
                nc.scalar.activation(o[:], u[:], AF.Copy, bias=-C)
            else:
                nc.vector.tensor_scalar_add(o[:], u[:], -C)
            nc.sync.dma_start(out[b], o[:])


def _get_nc(alpha, r, delta, s, eps):
    key = (alpha, r, delta, s, eps)
    if key not in _nc_cache:
        _nc_cache[key] = build_nc(alpha, r, delta, s, eps)
    return _nc_cache[key]


def make_in_maps(data: np.ndarray) -> list[dict]:
    """Shard the full [B,F,T] input into per-core input maps (T-sharding)."""
    data16 = data.astype(NPBF16)
    in_maps = []
    for c in range(NCORES):
        t0, t1 = c * TLOC, (c + 1) * TLOC
        xs_c = np.ascontiguousarray(data16[1:, :, t0:t1])
        x0h_c = np.zeros((F, TH), NPBF16)
        lo = max(0, t0 - HALO)
        x0h_c[:, HALO - (t0 - lo) :] = data16[0, :, lo:t1]
        in_maps.append({"xs": xs_c, "x0h": x0h_c})
    return in_maps


def kernel(data, alpha, r, delta, s, eps, _trace=False):
    data = np.ascontiguousarray(np.asarray(data, dtype=np.float32))
    assert data.shape == (B, F, T), data.shape
    a, rv, dv, sv, ev = (float(np.asarray(v)) for v in (alpha, r, delta, s, eps))
    nc = _get_nc(a, rv, dv, sv, ev)
    in_maps = make_in_maps(data)
    res = run_bass_kernel_spmd(nc, in_maps, list(range(NCORES)), trace=_trace)
    outp = np.empty((B, F, T), np.float32)
    for c in range(NCORES):
        outp[:, :, c * TLOC : (c + 1) * TLOC] = np.asarray(
            res.results[c]["out"]
        ).astype(np.float32)
    if _trace:
        return outp, res
    return outp


# revision 9
# speedup vs baseline: 1.2324x; 1.2324x over previous
"""PCEN kernel for Trainium2, sharded over the time axis across 8 NeuronCores.

Strategy:
  - data is [B=8, F=128, T=16384] fp32. Each core owns a T-slice of 2048 cols
    (all batches). Inputs are downcast to bf16 on the host and outputs are
    shipped back as bf16 (rel tolerance is 2e-2; bf16 rounding is ~2e-3 max),
    halving DMA traffic: ~8.5MB/core vs ~17MB -> DMA roofline ~22us.
  - The EMA smoother M uses only batch 0. Each core computes M for its own
    slice independently via a 32-col halo: contributions older than 32 steps
    are attenuated by (1-s)^32 ~ 2e-10, far below the 2e-2 tolerance. The
    scan itself is one native TensorTensorScan DVE instruction (fp32 state).
  - R = 1/M^alpha = exp(-alpha*ln(M+eps)) on the ACT engine (<=2 ULP tables;
    the Reciprocal activation is banned for accuracy).
  - Per batch b: E = x*R (DVE bf16 2x mode), U = sqrt(E + delta) (ACT fp32),
    out = U - delta^r (DVE tensor_scalar, fp32 in / bf16 out). The subtract
    must read fp32 U: near-zero outputs cancel (U ~ sqrt(delta)), so a bf16
    U would blow up the relative error. GPSIMD is excluded from the hot
    path: its tensor_scalar runs at ~9 G elem/s vs DVE's ~240 G elem/s.
  - All input DMAs are queued upfront on the sync (SP) HWDGE ring, x0-halo
    first then batches in compute order: the ring drains FIFO, so arrivals
    are staggered to match the compute pipeline instead of all completing
    together under round-robin.
"""

import sys

if "/opt/trn_rl_repo" not in sys.path:
    sys.path.insert(0, "/opt/trn_rl_repo")

from contextlib import ExitStack

import ml_dtypes
import numpy as np

import concourse.bass as bass
import concourse.mybir as mybir
import concourse.tile as tile
from concourse import bacc
from concourse.bass_utils import run_bass_kernel_spmd

B, F, T = 8, 128, 16384
NCORES = 8
TLOC = T // NCORES  # 2048 cols per core
HALO = 32  # scan warmup; (1-s)^32 ~ 2e-10 << 2e-2 tolerance
TH = TLOC + HALO
F32 = mybir.dt.float32
BF16 = mybir.dt.bfloat16
NPBF16 = ml_dtypes.bfloat16

_nc_cache: dict = {}


def build_nc(
    alpha: float,
    r: float,
    delta: float,
    s: float,
    eps: float,
    nbatch: int = B,
    tloc: int = TLOC,
    halo: int = HALO,
    reps: int = 1,
) -> bass.Bass:
    r_abs = abs(r)
    # constant subtracted at the end: delta ** |r|
    C = float(np.float32(delta) ** np.float32(r_abs))
    AF = mybir.ActivationFunctionType
    OP = mybir.AluOpType
    th = tloc + halo

    # Bacc (not raw Bass): its compile() lowers multi-sem waits into separate
    # sequencer instructions; the DMA/scan ISA structs hold only one wait.
    nc = bacc.Bacc("TRN2", target_bir_lowering=False, debug=False, num_devices=NCORES)

    xs = nc.dram_tensor("xs", [nbatch - 1, F, tloc], BF16, kind="ExternalInput").ap()
    x0h = nc.dram_tensor("x0h", [F, th], BF16, kind="ExternalInput").ap()
    out = nc.dram_tensor("out", [nbatch, F, tloc], BF16, kind="ExternalOutput").ap()
    # scratch target for the x0h-priority gate DMA (see _phase_ab).
    # One contiguous 512B row from a single partition: descriptors below
    # 512B do read-modify-write at ~2-5us each (128 of them wrecked the
    # DMA queues when this was a [F, 1] column).
    gate = nc.dram_tensor("gate", [1, 256], BF16, kind="ExternalOutput").ap()

    with ExitStack() as ctx:
        tc = ctx.enter_context(tile.TileContext(nc))
        scanp = ctx.enter_context(tc.tile_pool(name="scan", bufs=1))
        xpool = ctx.enter_context(tc.tile_pool(name="x", bufs=nbatch - 1))
        epool = ctx.enter_context(tc.tile_pool(name="e", bufs=3))
        upool = ctx.enter_context(tc.tile_pool(name="u", bufs=3))
        # one buf per batch: the out-DMA then has no slot-reuse (WAR) wait,
        # keeping it within the DMA struct's single sync-wait slot
        opool = ctx.enter_context(tc.tile_pool(name="o", bufs=nbatch))

        # reps>1 repeats the whole computation back-to-back (benchmarking
        # only: isolates device time from host/axon dispatch overhead)
        for _rep in range(reps):
            _phase_ab(nc, tc, scanp, xpool, epool, upool, opool,
                      xs, x0h, out, gate, nbatch, tloc, halo, th,
                      s, eps, alpha, delta, r_abs, C, AF, OP)
    nc.compile()
    return nc


def _phase_ab(nc, tc, scanp, xpool, epool, upool, opool,
              xs, x0h, out, gate, nbatch, tloc, halo, th,
              s, eps, alpha, delta, r_abs, C, AF, OP):
        # ---- input DMAs, x0h prioritized ----
        x0t = scanp.tile([F, th], BF16, tag="x0t")
        nc.sync.dma_start(x0t[:], x0h[:])
        # gate: a single-partition SBUF->DRAM copy of x0t makes the sync
        # sequencer wait for the x0h DMA to land before enqueueing the batch
        # loads below, so x0h gets the full SDMA bandwidth (the scan is the
        # critical path) instead of round-robining with 3.7MB of batch input.
        nc.sync.dma_start(gate[:], x0t[:1, :256])
        xtiles = []
        for b in range(1, nbatch):
            xbt = xpool.tile([F, tloc], BF16, tag="xb")
            nc.sync.dma_start(xbt[:], xs[b - 1])
            xtiles.append(xbt)

        # per-kernel const-bias tiles (activation() requires non-Copy biases
        # as APs); tile-pool deps replace the global all_engine_barrier the
        # shared const-AP registry would need.
        eps_t = scanp.tile([F, 1], F32, tag="epsc")
        nc.vector.memset(eps_t[:], float(eps))
        delta_t = scanp.tile([F, 1], F32, tag="deltac")
        nc.vector.memset(delta_t[:], float(delta))

        # ---- Phase A: EMA scan on batch 0 slice (with halo) ----
        # single decay column broadcast along the free dim (step-0 AP):
        # replaces a [F, th] memset on the scan's critical path
        dcol = scanp.tile([F, 1], BF16, tag="dcol")
        nc.vector.memset(dcol[:], 1.0 - s)
        # 1-col probe copy on the vector engine carries the DMA-completion
        # wait, so the scan instruction itself needs no cross-engine sync
        # waits (the TensorTensorScan ISA struct has too few wait slots).
        probe = scanp.tile([F, 1], BF16, tag="probe")
        nc.vector.tensor_copy(probe[:], x0t[:, :1])
        m = scanp.tile([F, th], F32, tag="m")
        # scan on raw x: state = (1-s)*state + x  => true EMA m = s*state.
        # The s factor is folded into the Ln activation's input scale below.
        dbc, _ = bass.broadcast_tensor_aps(dcol[:], x0t[:])
        nc.vector.tensor_tensor_scan(m[:], dbc, x0t[:], 0.0, OP.mult, OP.add)
        # R = exp(-alpha * ln(s*m' + eps)) == (M+eps)^-alpha  (M+eps > 0 always)
        lnm = scanp.tile([F, tloc], F32, tag="lnm")
        nc.scalar.activation(lnm[:], m[:, halo:], AF.Ln, bias=eps_t[:], scale=float(s))
        rr = scanp.tile([F, tloc], BF16, tag="rr")
        nc.scalar.activation(rr[:], lnm[:], AF.Exp, scale=-float(alpha))

        # ---- Phase B: per-batch elementwise PCEN ----
        for b in range(nbatch):
            if b == 0:
                xb = x0t[:, halo:]  # batch 0 slice already on chip
            else:
                xb = xtiles[b - 1][:]
            e = epool.tile([F, tloc], BF16, tag="e")
            nc.vector.tensor_mul(e[:], xb, rr[:])  # E = x / M^alpha
            u = upool.tile([F, tloc], F32, tag="u")
            if r_abs == 0.5:
                nc.scalar.activation(u[:], e[:], AF.Sqrt, bias=delta_t[:])
            else:
                lne = upool.tile([F, tloc], F32, tag="lne")
                nc.scalar.activation(lne[:], e[:], AF.Ln, bias=delta_t[:])
                nc.scalar.activation(u[:], lne[:], AF.Exp, scale=float(r_abs))
            o = opool.tile([F, tloc], BF16, tag="o")
            if b == nbatch - 1:
                # last batch's subtract on ACT (Copy is table-free): balances
                # DVE (8 muls + 7 subs) vs ACT (8 sqrts + 1 copy) and avoids
                # a DVE->ACT->DVE ping-pong on the drain tail.
                nc.scalar.activation(o[:], u[:], AF.Copy, bias=-C)
            else:
                nc.vector.tensor_scalar_add(o[:], u[:], -C)
            nc.sync.dma_start(out[b], o[:])


def _get_nc(alpha, r, delta, s, eps):
    key = (alpha, r, delta, s, eps)
    if key not in _nc_cache:
        _nc_cache[key] = build_nc(alpha, r, delta, s, eps)
    return _nc_cache[key]


def make_in_maps(data: np.ndarray) -> list[dict]:
    """Shard the full [B,F,T] input into per-core input maps (T-sharding)."""
    data16 = data.astype(NPBF16)
    in_maps = []
    for c in range(NCORES):
        t0, t1 = c * TLOC, (c + 1) * TLOC
        xs_c = np.ascontiguousarray(data16[1:, :, t0:t1])
        x0h_c = np.zeros((F, TH), NPBF16)
        lo = max(0, t0 - HALO)
        x0h_c[:, HALO - (t0 - lo) :] = data16[0, :, lo:t1]
        in_maps.append({"xs": xs_c, "x0h": x0h_c})
    return in_maps


def kernel(data, alpha, r, delta, s, eps, _trace=False):
    data = np.ascontiguousarray(np.asarray(data, dtype=np.float32))
    assert data.shape == (B, F, T), data.shape
    a, rv, dv, sv, ev = (float(np.asarray(v)) for v in (alpha, r, delta, s, eps))
    nc = _get_nc(a, rv, dv, sv, ev)
    in_maps = make_in_maps(data)
    res = run_bass_kernel_spmd(nc, in_maps, list(range(NCORES)), trace=_trace)
    outp = np.empty((B, F, T), np.float32)
    for c in range(NCORES):
        outp[:, :, c * TLOC : (c + 1) * TLOC] = np.asarray(
            res.results[c]["out"]
        ).astype(np.float32)
    if _trace:
        return outp, res
    return outp
